# revision 13
# baseline (speedup 1.0000x reference)
"""Batched ICP (16x2048x3, 10 iters) on 8 Trainium2 NeuronCores.

Data-parallel over the batch dim: 2 batches per core, no cross-core comm.

Per ICP iteration, per chunk of 128 src points:
  - PE: 4 row-packed fp32r matmuls -> score[n,m] = cur.tgt - 0.5|tgt|^2 (PSUM)
  - DVE: tensor_reduce(max, negate) per half; fused min+delta -> relu bias
  - ACT: W = relu(score + (-rowmax + b))  -> soft one-hot weights in SBUF
  - PE: P += curaug^T @ W  (4 col-groups packed into one PSUM bank)
Tail per iteration: Hraw = P^T-chunks x tgtaug-chunks via small matmuls,
rank-1 centering, bit-trick rsqrt for the scale, Newton-Schulz polar iteration
(tiny matmuls only) for the Kabsch rotation (det(H)>0 holds for this data),
point transform, layout rebuild via PE transposes. No gathers or index math.
"""
import os
import numpy as np

os.environ.setdefault("MYCRO_LOCAL_CACHE", "1")

RELU_B = 1e-4          # relu ramp width (soft one-hot)
NS_ITERS = 9           # Newton-Schulz iterations
ICP_ITERS = int(os.environ.get("ICP_ITERS_OVERRIDE", "10"))
N = 2048
NCHUNK = 16
CORES = 8
BPC = 2
MM_DT = os.environ.get("ICP_MM_DT", "float32")  # dtype for score/P matmuls


def _build(nc):
    import concourse.mybir as mybir
    import concourse.tile as tile
    import contextlib

    dt = mybir.dt
    f32 = dt.float32
    mmdt = getattr(dt, MM_DT)
    AF = mybir.ActivationFunctionType
    ALU = mybir.AluOpType
    AX = mybir.AxisListType

    def cast(ap):
        return ap.bitcast(mmdt) if mmdt != f32 else ap

    ins = {}
    outs = {}
    for b in range(BPC):
        ins[f"curchunk{b}"] = nc.declare_dram_parameter(f"curchunk{b}", [128, NCHUNK * 4], f32, isOutput=False)
        ins[f"curT{b}"] = nc.declare_dram_parameter(f"curT{b}", [128, N], f32, isOutput=False)
        ins[f"tgtchunk{b}"] = nc.declare_dram_parameter(f"tgtchunk{b}", [128, NCHUNK * 4], f32, isOutput=False)
        ins[f"tgtT{b}"] = nc.declare_dram_parameter(f"tgtT{b}", [128, N], f32, isOutput=False)
        outs[f"aligned{b}"] = nc.declare_dram_parameter(f"aligned{b}", [N, 3], f32, isOutput=True)
        outs[f"Rout{b}"] = nc.declare_dram_parameter(f"Rout{b}", [3, 3], f32, isOutput=True)
        outs[f"tout{b}"] = nc.declare_dram_parameter(f"tout{b}", [3, 1], f32, isOutput=True)
    ins["ident128"] = nc.declare_dram_parameter("ident128", [128, 128], f32, isOutput=False)
    ins["ident4rep"] = nc.declare_dram_parameter("ident4rep", [128, 4], f32, isOutput=False)
    ins["consts"] = nc.declare_dram_parameter("consts", [4, 8], f32, isOutput=False)

    with tile.TileContext(nc) as tc, contextlib.ExitStack() as ctx:
        sing = ctx.enter_context(tc.tile_pool(name="sing", bufs=1))
        wpool = ctx.enter_context(tc.tile_pool(name="wpool", bufs=4))
        rpool = ctx.enter_context(tc.tile_pool(name="rpool", bufs=4))
        tpool = ctx.enter_context(tc.tile_pool(name="tpool", bufs=2))
        psS = ctx.enter_context(tc.tile_pool(name="psS", bufs=3, space="PSUM"))
        psP = ctx.enter_context(tc.tile_pool(name="psP", bufs=1, space="PSUM"))

        ident128 = sing.tile([128, 128], f32, name="ident128", tag="ident128")
        nc.sync.dma_start(out=ident128, in_=ins["ident128"][:, :])
        ident4rep = sing.tile([128, 4], f32, name="ident4rep", tag="ident4rep")
        nc.sync.dma_start(out=ident4rep, in_=ins["ident4rep"][:, :])
        consts = sing.tile([4, 8], f32, name="consts", tag="consts")
        nc.sync.dma_start(out=consts, in_=ins["consts"][:, :])
        eye3 = consts[0:3, 0:3]
        eye3_15 = consts[0:3, 3:6]
        ones31 = consts[0:3, 6:7]
        one11 = consts[0:1, 6:7]
        ones14 = sing.tile([1, 4], f32, name="ones14", tag="ones14")
        nc.gpsimd.memset(ones14, 1.0)
        delta_t = sing.tile([128, 1], f32, name="delta", tag="delta")
        nc.gpsimd.memset(delta_t, RELU_B)

        st = {}
        for b in range(BPC):
            s = st[b] = {}
            s["curchunk"] = sing.tile([128, NCHUNK * 4], f32, name=f"curchunk{b}", tag=f"curchunk{b}")
            nc.sync.dma_start(out=s["curchunk"], in_=ins[f"curchunk{b}"][:, :])
            s["tgtchunk"] = sing.tile([128, NCHUNK * 4], f32, name=f"tgtchunk{b}", tag=f"tgtchunk{b}")
            nc.sync.dma_start(out=s["tgtchunk"], in_=ins[f"tgtchunk{b}"][:, :])
            s["curT"] = sing.tile([128, N], f32, name=f"curT{b}", tag=f"curT{b}")
            s["tgtT"] = sing.tile([128, N], f32, name=f"tgtT{b}", tag=f"tgtT{b}")
            nc.sync.dma_start(out=s["curT"], in_=ins[f"curT{b}"][:, :])
            nc.sync.dma_start(out=s["tgtT"], in_=ins[f"tgtT{b}"][:, :])
            s["Rtot"] = sing.tile([3, 3], f32, name=f"Rtot{b}", tag=f"Rtot{b}")
            nc.scalar.copy(out=s["Rtot"], in_=eye3)
            s["ttot"] = sing.tile([3, 1], f32, name=f"ttot{b}", tag=f"ttot{b}")
            nc.gpsimd.memset(s["ttot"], 0.0)
            s["Raug"] = sing.tile([4, 3], f32, name=f"Raug{b}", tag=f"Raug{b}")

        def chunks_phase(b):
            s = st[b]
            P_ps = psP.tile([128, 512], f32, name=f"P{b}", tag=f"P{b}")
            for c in range(NCHUNK):
                S0 = psS.tile([128, 1024], f32, name="S", tag="S")
                S1 = psS.tile([128, 1024], f32, name="S", tag="S")
                for t in range(4):
                    Sd = S0 if t < 2 else S1
                    nc.tensor.matmul(
                        Sd[:, (t % 2) * 512:(t % 2) * 512 + 512],
                        cast(s["curT"][32 * t:32 * t + 4, c * 128:(c + 1) * 128]),
                        cast(s["tgtT"][32 * t:32 * t + 4, t * 512:(t + 1) * 512]),
                        start=True, stop=True,
                        tile_position=(32 * t, 0),
                    )
                pm0 = rpool.tile([128, 1], f32, name="pm0", tag="pm0")
                pm1 = rpool.tile([128, 1], f32, name="pm1", tag="pm1")
                nc.vector.tensor_reduce(out=pm0, in_=S0, axis=AX.X, op=ALU.max, negate=True)
                nc.vector.tensor_reduce(out=pm1, in_=S1, axis=AX.X, op=ALU.max, negate=True)
                bias = rpool.tile([128, 1], f32, name="bias", tag="bias")
                nc.vector.scalar_tensor_tensor(
                    out=bias, in0=pm0, scalar=pm1, in1=delta_t,
                    op0=ALU.min, op1=ALU.add,
                )
                W0 = wpool.tile([128, 1024], f32, name="W", tag="W")
                W1 = wpool.tile([128, 1024], f32, name="W", tag="W")
                nc.scalar.activation(out=W0, in_=S0, func=AF.Relu, bias=bias, scale=1.0)
                nc.scalar.activation(out=W1, in_=S1, func=AF.Relu, bias=bias, scale=1.0)
                for t in range(4):
                    Wd = W0 if t < 2 else W1
                    nc.tensor.matmul(
                        P_ps[32 * t:32 * t + 4, :],
                        cast(s["curchunk"][:, c * 4:c * 4 + 4]),
                        cast(Wd[:, (t % 2) * 512:(t % 2) * 512 + 512]),
                        start=(c == 0), stop=(c == NCHUNK - 1),
                        tile_position=(0, 32 * t),
                        skip_group_check=True,
                    )
            return P_ps

        def tail_phase(b, k, P_ps):
            s = st[b]
            def pt(shape):
                return psP.tile(shape, f32, name=f"P{b}", tag=f"P{b}")
            def sbt(shape, tg):
                return tpool.tile(shape, f32, name=f"{tg}{b}", tag=f"{tg}{b}")

            Psb = sbt([128, 512], "Psb")
            for t in range(4):
                nc.scalar.copy(out=Psb[32 * t:32 * t + 4, :], in_=P_ps[32 * t:32 * t + 4, :])

            PT_ps = pt([128, 64])
            for c in range(NCHUNK):
                t = c // 4
                nc.tensor.transpose(
                    PT_ps[:, c * 4:c * 4 + 4],
                    Psb[32 * t:32 * t + 4, (c % 4) * 128:(c % 4) * 128 + 128],
                    ident4rep[32 * t:32 * t + 4, :],
                    tile_position=(32 * t, 0),
                )
            PTsb = sbt([128, 64], "PTsb")
            nc.vector.tensor_copy(out=PTsb, in_=PT_ps)

            H_ps = pt([4, 4])
            for c in range(NCHUNK):
                nc.tensor.matmul(
                    H_ps, PTsb[:, c * 4:c * 4 + 4], s["tgtchunk"][:, c * 4:c * 4 + 4],
                    start=(c == 0), stop=(c == NCHUNK - 1),
                )
            Hraw = sbt([4, 4], "Hraw")
            nc.scalar.copy(out=Hraw, in_=H_ps)
            HrawT_ps = pt([4, 4])
            nc.tensor.transpose(HrawT_ps, Hraw, ident4rep[0:4, :])
            HrawT = sbt([4, 4], "HrawT")
            nc.scalar.copy(out=HrawT, in_=HrawT_ps)

            scur_row_ps = pt([1, 4])
            nc.tensor.transpose(scur_row_ps, Hraw[0:4, 3:4], ident4rep[0:4, :])
            scur_row = sbt([1, 4], "scurrow")
            nc.scalar.copy(out=scur_row, in_=scur_row_ps)
            scorr_row_ps = pt([1, 4])
            nc.tensor.transpose(scorr_row_ps, HrawT[0:4, 3:4], ident4rep[0:4, :])
            scorr_row = sbt([1, 4], "scorrrow")
            nc.scalar.copy(out=scorr_row, in_=scorr_row_ps)

            invn = sbt([1, 1], "invn")
            nc.vector.reciprocal(out=invn, in_=scur_row[0:1, 3:4])
            in4_ps = pt([4, 1])
            nc.tensor.matmul(in4_ps, ones14, invn, start=True, stop=True)
            invn4 = sbt([4, 1], "invn4")
            nc.scalar.copy(out=invn4, in_=in4_ps)
            invn4n = sbt([4, 1], "invn4n")
            nc.scalar.activation(out=invn4n, in_=in4_ps, func=AF.Copy, scale=-1.0)
            scorr_row_n = sbt([1, 3], "scorrrown")
            nc.vector.tensor_scalar(scorr_row_n, scorr_row[0:1, 0:3], invn4n[0:1, 0:1], None, ALU.mult)
            scur_row_n = sbt([1, 3], "scurrown")
            nc.vector.tensor_scalar(scur_row_n, scur_row[0:1, 0:3], invn4n[0:1, 0:1], None, ALU.mult)

            Hc_ps = pt([3, 3])
            nc.tensor.matmul(Hc_ps, scur_row[0:1, 0:3], scorr_row_n, start=True, stop=False)
            nc.tensor.matmul(Hc_ps, eye3, Hraw[0:3, 0:3], start=False, stop=True)
            Hc = sbt([3, 3], "Hc")
            nc.scalar.copy(out=Hc, in_=Hc_ps)
            HcT_ps = pt([3, 3])
            nc.tensor.matmul(HcT_ps, scorr_row[0:1, 0:3], scur_row_n, start=True, stop=False)
            nc.tensor.matmul(HcT_ps, eye3, HrawT[0:3, 0:3], start=False, stop=True)
            HcT = sbt([3, 3], "HcT")
            nc.scalar.copy(out=HcT, in_=HcT_ps)

            Hsq = sbt([3, 3], "Hsq")
            nc.vector.tensor_tensor(out=Hsq, in0=Hc, in1=Hc, op=ALU.mult)
            s3 = sbt([3, 1], "s3")
            nc.vector.tensor_reduce(out=s3, in_=Hsq, axis=AX.X, op=ALU.add)
            ssq_ps = pt([1, 1])
            nc.tensor.matmul(ssq_ps, s3, ones31, start=True, stop=True)
            ssq = sbt([1, 1], "ssq")
            nc.vector.tensor_copy(out=ssq, in_=ssq_ps)
            yb0 = sbt([1, 1], "yb0")
            nc.vector.tensor_scalar(yb0.bitcast(dt.int32), ssq.bitcast(dt.int32), 1, None, ALU.arith_shift_right)
            yb = sbt([1, 1], "yb")
            nc.vector.tensor_scalar(yb.bitcast(dt.int32), yb0.bitcast(dt.int32), 0x5F3759DF, -1, ALU.subtract, ALU.mult)
            y4_ps = pt([4, 1])
            nc.tensor.matmul(y4_ps, ones14, yb, start=True, stop=True)
            y4 = sbt([4, 1], "y4")
            nc.scalar.copy(out=y4, in_=y4_ps)

            X = sbt([3, 3], "X")
            nc.vector.tensor_scalar(X, HcT, y4[0:3, 0:1], None, ALU.mult)
            Z = sbt([3, 3], "Z")
            nc.vector.tensor_scalar(Z, Hc, y4[0:3, 0:1], None, ALU.mult)
            for _ in range(NS_ITERS):
                G_ps = pt([3, 3])
                nc.tensor.matmul(G_ps, X, X, start=True, stop=True)
                Gn = sbt([3, 3], "Gn")
                nc.scalar.activation(out=Gn, in_=G_ps, func=AF.Copy, scale=-0.5)
                Xn_ps = pt([3, 3])
                nc.tensor.matmul(Xn_ps, Z, Gn, start=True, stop=False)
                nc.tensor.matmul(Xn_ps, eye3_15, X, start=False, stop=True)
                Zn_ps = pt([3, 3])
                nc.tensor.matmul(Zn_ps, Gn, Z, start=True, stop=False)
                nc.tensor.matmul(Zn_ps, eye3_15, Z, start=False, stop=True)
                X = sbt([3, 3], "X")
                nc.scalar.copy(out=X, in_=Xn_ps)
                Z = sbt([3, 3], "Z")
                nc.scalar.copy(out=Z, in_=Zn_ps)
            # X = R, Z = R^T

            csn_col = sbt([3, 1], "csncol")
            nc.vector.tensor_scalar(csn_col, Hraw[0:3, 3:4], invn4n[0:3, 0:1], None, ALU.mult)
            ct_col = sbt([3, 1], "ctcol")
            nc.vector.tensor_scalar(ct_col, HrawT[0:3, 3:4], invn4[0:3, 0:1], None, ALU.mult)
            tcol_ps = pt([3, 1])
            nc.tensor.matmul(tcol_ps, Z, csn_col, start=True, stop=False)
            nc.tensor.matmul(tcol_ps, eye3, ct_col, start=False, stop=True)
            tcol = sbt([3, 1], "tcol")
            nc.scalar.copy(out=tcol, in_=tcol_ps)

            # Raug^T = [R | t] rows at p0-2, then transpose -> Raug [4,3]
            RaugT = sbt([3, 4], "RaugT")
            nc.scalar.copy(out=RaugT[:, 0:3], in_=X)
            nc.scalar.copy(out=RaugT[:, 3:4], in_=tcol)
            Raug_ps = pt([4, 3])
            nc.tensor.transpose(Raug_ps, RaugT, ident4rep[0:3, 0:3])
            nc.scalar.copy(out=s["Raug"], in_=Raug_ps)

            Rt_ps = pt([3, 3])
            nc.tensor.matmul(Rt_ps, Z, s["Rtot"], start=True, stop=True)
            nc.scalar.copy(out=s["Rtot"], in_=Rt_ps)
            tt_ps = pt([3, 1])
            nc.tensor.matmul(tt_ps, Z, s["ttot"], start=True, stop=False)
            nc.tensor.matmul(tt_ps, eye3, tcol, start=False, stop=True)
            nc.scalar.copy(out=s["ttot"], in_=tt_ps)

            TF_ps = pt([128, 48])
            for c in range(NCHUNK):
                nc.tensor.matmul(
                    TF_ps[:, c * 3:c * 3 + 3],
                    s["curT"][0:4, c * 128:(c + 1) * 128],
                    s["Raug"],
                    start=True, stop=True,
                    skip_group_check=True,
                )
            nc.scalar.copy(
                out=s["curchunk"].rearrange("p (c q) -> p c q", q=4)[:, :, 0:3],
                in_=TF_ps.rearrange("p (c q) -> p c q", q=3),
            )

            if k == ICP_ITERS - 1:
                nc.sync.dma_start(
                    out=outs[f"aligned{b}"][:, :].rearrange("(c p) d -> p c d", p=128),
                    in_=s["curchunk"].rearrange("p (c q) -> p c q", q=4)[:, :, 0:3],
                )
                nc.sync.dma_start(out=outs[f"Rout{b}"][:, :], in_=s["Rtot"])
                nc.sync.dma_start(out=outs[f"tout{b}"][:, :], in_=s["ttot"])
            else:
                for q in range(4):
                    CT_ps = pt([4, 512])
                    for j in range(4):
                        c = q * 4 + j
                        nc.tensor.transpose(
                            CT_ps[:, j * 128:j * 128 + 128],
                            s["curchunk"][:, c * 4:c * 4 + 4],
                            ident128,
                        )
                    nc.scalar.copy(out=s["curT"][0:4, q * 512:(q + 1) * 512], in_=CT_ps)
                for t in range(1, 4):
                    nc.sync.dma_start(out=s["curT"][32 * t:32 * t + 4, :], in_=s["curT"][0:4, :])

        for k in range(ICP_ITERS):
            for b in range(BPC):
                P_ps = chunks_phase(b)
                tail_phase(b, k, P_ps)

    return nc


def _consts_np():
    c = np.zeros((4, 8), np.float32)
    c[0:3, 0:3] = np.eye(3, dtype=np.float32)
    c[0:3, 3:6] = 1.5 * np.eye(3, dtype=np.float32)
    c[0:3, 6:7] = 1.0
    return c


def _prep_core_inputs(src2, tgt2, consts_np):
    m = {}
    for b in range(BPC):
        src = src2[b].astype(np.float32)
        tgt = tgt2[b].astype(np.float32)
        ones = np.ones((N, 1), np.float32)
        cur_aug = np.concatenate([src, ones], 1)
        tgt_aug = np.concatenate([tgt, ones], 1)
        tgt_sc = np.concatenate([tgt, -0.5 * np.sum(tgt * tgt, 1, keepdims=True)], 1)
        m[f"curchunk{b}"] = np.ascontiguousarray(
            cur_aug.reshape(NCHUNK, 128, 4).transpose(1, 0, 2).reshape(128, NCHUNK * 4))
        m[f"tgtchunk{b}"] = np.ascontiguousarray(
            tgt_aug.reshape(NCHUNK, 128, 4).transpose(1, 0, 2).reshape(128, NCHUNK * 4))
        curT = np.zeros((128, N), np.float32)
        tgtT = np.zeros((128, N), np.float32)
        for t in range(4):
            curT[32 * t:32 * t + 4, :] = cur_aug.T
            tgtT[32 * t:32 * t + 4, :] = tgt_sc.T
        m[f"curT{b}"] = curT
        m[f"tgtT{b}"] = tgtT
    m["ident128"] = np.eye(128, dtype=np.float32)
    id4 = np.zeros((128, 4), np.float32)
    for t in range(4):
        id4[32 * t:32 * t + 4, :] = np.eye(4, dtype=np.float32)
    m["ident4rep"] = id4
    m["consts"] = consts_np
    return m


def kernel(src_points, tgt_points):
    import concourse.bacc as bacc
    from concourse.bass_utils import run_bass_kernel_spmd

    src = np.asarray(src_points, dtype=np.float32)
    tgt = np.asarray(tgt_points, dtype=np.float32)
    B = src.shape[0]
    assert B == CORES * BPC and src.shape[1] == N

    nc = bacc.Bacc(None, target_bir_lowering=False)
    _build(nc)
    nc.compile()

    consts_np = _consts_np()
    in_maps = [
        _prep_core_inputs(src[i * BPC:(i + 1) * BPC], tgt[i * BPC:(i + 1) * BPC], consts_np)
        for i in range(CORES)
    ]
    res = run_bass_kernel_spmd(nc, in_maps, core_ids=list(range(CORES)),
                               trace=bool(int(os.environ.get("ICP_TRACE", "0"))))

    aligned = np.empty((B, N, 3), np.float32)
    R = np.empty((B, 3, 3), np.float32)
    t = np.empty((B, 3), np.float32)
    for i in range(CORES):
        out = res.results[i]
        for b in range(BPC):
            g = i * BPC + b
            aligned[g] = out[f"aligned{b}"]
            R[g] = out[f"Rout{b}"]
            t[g] = out[f"tout{b}"].reshape(3)
    kernel._last_exec_time_ns = res.exec_time_ns
    return aligned, (R, t)
